# revision 1
# baseline (speedup 1.0000x reference)
"""Radius-count kernel (torch.cdist + threshold + sum) for Trainium2, 8 cores.

counts[n] = #{ m : ||padding[m] - pointcloud[n]|| <= 0.5 }

Strategy
--------
d^2(n,m) <= 0.25  <=>  q(n,m) = 0.25 - |a_m|^2 - |b_n|^2 + 2 a_m.b_n >= 0.

q is a bilinear form, so each (n-tile, m-chunk) block of q is one small-K
matmul on the PE array.  To get fp32-grade accuracy at bf16 matmul speed,
every fp32 operand is decomposed exactly into 3 bf16 pieces (8 mantissa
bits each, power-of-two scales), and the matmul contracts over all piece
cross-products except the negligible lo*lo term: K = 30 rows.  Every
product of two pieces is exact in fp32, so the only error vs. the jax
reference is fp32 accumulation-order rounding (~1e-7 relative on d^2).

Each core handles 25000/8 = 3125 padding points (padded to 3136 columns)
against all 20000 pointcloud points (157 tiles of 128 partitions).  The
threshold+count epilogue is the bottleneck (only ScalarE and VectorE can
read PSUM; DMA has no PSUM route on TRN2), so it is split across both
per-element engines, each consuming a whole PSUM block in ONE instruction:
 - ScalarE: activation(Sign) with fused free-axis accumulation (sum of +-1)
 - VectorE: tensor_scalar(is_ge 0) with fused add-reduction (sum of 0/1)
Per-core partial counts come back as two [128, 314] f32 tensors; the host
combines and all-reduces them (80KB/core).  Timeline sim: 323.6us/core with
both epilogue engines ~96% busy; HW repeat-delta measurements 347-449us
(median ~412us; the repeat loop adds back-edge + IRAM-refetch per iteration,
so one-shot time is nearer the low end).

Design notes from the tuning loop (what was tried and rejected):
 - fp32 matmul: 4 cyc/row on TRN2 PE (~940us) -> replaced by the exact
   3x-bf16 decomposition at 1 cyc/row.
 - GPSIMD as a 3rd epilogue engine: dead - DMA has no PSUM route on TRN2,
   and only ScalarE/VectorE can read PSUM.
 - In-place epilogue writes to PSUM: measured ~40% SLOWER on HW (same-bank
   read+write port contention); rotating SBUF scratch wins.
 - Merged two-subtile ACT ops (one instr per tile, saves the 187ns
   accumulator-read): every variant exceeds the 8 PSUM banks needed for
   double buffering, or unbalances the engines.  Bank-constrained optimum
   is 2 ops/engine/tile with 2-bank tiles, bufs=2 on both pools.
 - Spatial pruning (Morton / x-sorted block skipping): only 8-24% of blocks
   are skippable at the PSUM-op granularity (~800 m-cols), and per-core
   skip schedules break the single-program SPMD contract.
 - is_ge+reduce on ScalarE (would skip the 187ns accumulator read): not
   possible - ScalarE only runs tensor ops expressible as activations,
   and is_ge is not; Sign+accum_out is its cheapest counting op.
 - Chunked lhs DMA for startup overlap: per-DMA overhead exceeds the
   win (sim: 327-332us vs 323.6us); single DMA kept.
"""

import numpy as np
import ml_dtypes

import os

N = 20000
M = 25000
NCORES = 8
NT = 157                 # n-tiles of 128 -> 20096 columns
NPAD = NT * 128
MS = M // NCORES         # 3125 padding points per core
# m-columns per PSUM block: CA via ScalarE(Sign), CD via VectorE(is_ge).
# Each must fit in 2 PSUM banks (<=1024 f32); 2 blocks of each per n-tile.
# Tuned on-HW: ScalarE(Sign) per-op overhead is ~3x VectorE's, so VectorE
# gets the larger chunk.  Both engines sit at ~96% busy in the timeline sim.
CA = int(os.environ.get("KRN_CA", "736"))
CD = int(os.environ.get("KRN_CD", "832"))
REPEAT = int(os.environ.get("KRN_REPEAT", "1"))  # timing-only: loop body R times
INPLACE_ACT = os.environ.get("KRN_INPLACE", "0") == "1"  # measured slower on HW; off
SCR_BUFS = int(os.environ.get("KRN_SCRBUFS", "3"))
DVE_FIRST = os.environ.get("KRN_DVEFIRST", "0") == "1"
TABLE_WARM = os.environ.get("KRN_TABLEWARM", "0") == "1"  # probed: table load already overlapped
MPAD = 2 * (CA + CD)     # 3136 >= 3125
K = 30                   # contraction rows
ACT_COLS = 2 * CA        # m-columns counted via Sign (+-1) per core

_BF = ml_dtypes.bfloat16
_PROGRAMS = {}           # repeat -> cached compiled Bass program
LAST_RESULTS = None      # BassKernelResults of the most recent run


def _split3(x):
    """Exact 3-way bf16 decomposition of fp32 data: x == p0+p1+p2 (up to
    ~2^-25 relative from a possible carry in the last piece)."""
    x = np.asarray(x, np.float32)
    p0 = x.astype(_BF).astype(np.float32)
    r = (x - p0).astype(np.float32)
    p1 = r.astype(_BF).astype(np.float32)
    r2 = (r - p1).astype(np.float32)
    p2 = r2.astype(_BF).astype(np.float32)
    return p0, p1, p2


def _norm2(p):
    """fp32 row norms with the same op order as jnp.sum(p*p, axis=1)."""
    pp = (p * p).astype(np.float32)
    return ((pp[:, 0] + pp[:, 1]) + pp[:, 2]).astype(np.float32)


def _row_plan(B, nb, one_l, A, s, one_r):
    """The K=30 contraction rows, smallest magnitude first (PSUM partial sums
    accumulate in row order; adding small terms first minimizes rounding).

    B[c][i] : lhs coordinate pieces (2*b_c piece i), per pointcloud point
    nb[i]   : -(|b|^2) piece i
    A[c][j] : rhs coordinate pieces (a_c piece j), per padding point
    s[i]    : (0.25 - |a|^2) piece i
    one_l/one_r : constant 1.0 rows
    """
    rows = []
    for c in range(3):
        rows.append((B[c][1], A[c][2]))
    for c in range(3):
        rows.append((B[c][2], A[c][1]))
    for c in range(3):
        rows.append((B[c][1], A[c][1]))
    for c in range(3):
        rows.append((B[c][0], A[c][2]))
    for c in range(3):
        rows.append((B[c][2], A[c][0]))
    rows.append((nb[2], one_r))
    rows.append((one_l, s[2]))
    for c in range(3):
        rows.append((B[c][0], A[c][1]))
    for c in range(3):
        rows.append((B[c][1], A[c][0]))
    rows.append((nb[1], one_r))
    rows.append((one_l, s[1]))
    for c in range(3):
        rows.append((B[c][0], A[c][0]))
    rows.append((nb[0], one_r))
    rows.append((one_l, s[0]))
    assert len(rows) == K
    return rows


def _build_operands(pointcloud, padding_shard):
    """lhs_t [K, NPAD] bf16 (pointcloud side), rhs [K, MPAD] bf16 (padding)."""
    lhs = _build_lhs(pointcloud)
    return lhs, _build_rhs(padding_shard)


def _build_lhs(pointcloud):
    b = np.asarray(pointcloud, np.float32)
    nb_full = -_norm2(b)
    B = []
    for c in range(3):
        p0, p1, p2 = _split3(b[:, c])
        B.append((2.0 * p0, 2.0 * p1, 2.0 * p2))  # exact in bf16
    nb = _split3(nb_full)
    one_l = np.ones(b.shape[0], np.float32)
    # rhs-side args are placeholders of matching length; only lhs rows used
    zero_r = (np.zeros(1, np.float32),) * 3
    rows = _row_plan(B, nb, one_l, [zero_r] * 3, zero_r, np.zeros(1, np.float32))
    lhs = np.zeros((K, NPAD), np.float32)
    nv = b.shape[0]
    for k, (lrow, _) in enumerate(rows):
        lhs[k, :nv] = lrow
    return lhs.astype(_BF)


def _build_rhs(padding_shard):
    a = np.asarray(padding_shard, np.float32)
    s_full = (np.float32(0.25) - _norm2(a)).astype(np.float32)
    A = []
    for c in range(3):
        A.append(_split3(a[:, c]))
    s = _split3(s_full)
    one_r = np.ones(a.shape[0], np.float32)
    zero_l = (np.zeros(1, np.float32),) * 3
    rows = _row_plan([zero_l] * 3, zero_l, np.zeros(1, np.float32), A, s, one_r)
    rhs = np.zeros((K, MPAD), np.float32)
    mv = a.shape[0]
    for k, (_, rrow) in enumerate(rows):
        rhs[k, :mv] = rrow
    # Padded m columns: q = -1 (never counted).  Row K-1 is (one_l, s[0]).
    rhs[:, mv:] = 0.0
    rhs[K - 1, mv:] = -1.0
    return rhs.astype(_BF)


def _get_program(repeat=None):
    if repeat is None:
        repeat = REPEAT
    if repeat in _PROGRAMS:
        return _PROGRAMS[repeat]

    import concourse.bacc as bacc
    import concourse.mybir as mybir
    import concourse.tile as tile

    nc = bacc.Bacc("TRN2", target_bir_lowering=False, debug=False,
                   enable_asserts=False, num_devices=NCORES)
    f32 = mybir.dt.float32
    bf16 = mybir.dt.bfloat16
    lhs_d = nc.dram_tensor("lhs_t", [K, NPAD], bf16, kind="ExternalInput").ap()
    rhs_d = nc.dram_tensor("rhs", [K, MPAD], bf16, kind="ExternalInput").ap()
    act_d = nc.dram_tensor("actsum", [128, 2 * NT], f32, kind="ExternalOutput").ap()
    dve_d = nc.dram_tensor("dvesum", [128, 2 * NT], f32, kind="ExternalOutput").ap()

    with tile.TileContext(nc) as tc:
        with tc.tile_pool(name="const", bufs=1) as cpool, \
             tc.tile_pool(name="psA", bufs=2, space="PSUM") as psA, \
             tc.tile_pool(name="psB", bufs=2, space="PSUM") as psB, \
             tc.tile_pool(name="scr", bufs=SCR_BUFS) as scr, \
             tc.tile_pool(name="accp", bufs=1) as accp:
            lhs_sb = cpool.tile([K, NPAD], bf16)
            # chunked load probed in sim: overhead exceeds the startup win;
            # default stays a single DMA (KRN_LHSCHUNKS=1)
            NCH = int(os.environ.get("KRN_LHSCHUNKS", "1"))
            cw = NPAD // NCH
            for c in range(NCH):
                lo = c * cw
                hi = NPAD if c == NCH - 1 else (c + 1) * cw
                nc.sync.dma_start(out=lhs_sb[:, lo:hi], in_=lhs_d[:, lo:hi])
            rhs_sb = cpool.tile([K, MPAD], bf16)
            nc.sync.dma_start(out=rhs_sb, in_=rhs_d)
            bias_sb = cpool.tile([128, 1], f32)
            nc.vector.memset(bias_sb, 1e-30)
            if TABLE_WARM:
                # absorb the ~2.7us Sign table load under the input DMA
                warm_sb = cpool.tile([128, 1], f32)
                nc.scalar.activation(warm_sb, bias_sb,
                                     mybir.ActivationFunctionType.Sign,
                                     bias=bias_sb)

            act_sb = accp.tile([128, 2 * NT], f32)
            dve_sb = accp.tile([128, 2 * NT], f32)

            def fill_psum(ps, lt, c0, width):
                """Fill a [128, width] PSUM tile from rhs columns [c0, c0+width)."""
                o = 0
                while o < width:
                    w = min(512, width - o)
                    nc.tensor.matmul(ps[:, o:o + w], lt,
                                     rhs_sb[:, c0 + o:c0 + o + w],
                                     start=True, stop=True)
                    o += w

            def body():
                for t in range(NT):
                    lt = lhs_sb[:, t * 128:(t + 1) * 128]
                    for j in range(2):
                        base = j * (CA + CD)
                        col = 2 * t + j
                        if DVE_FIRST:
                            pb = psB.tile([128, CD], f32)
                            fill_psum(pb, lt, base + CA, CD)
                            sv = scr.tile([128, CD], f32, tag="sv")
                            nc.vector.tensor_scalar(
                                sv, pb, 0.0, 0.0,
                                op0=mybir.AluOpType.is_ge, op1=mybir.AluOpType.add,
                                accum_out=dve_sb[:, col:col + 1])
                        pa = psA.tile([128, CA], f32)
                        fill_psum(pa, lt, base, CA)
                        if INPLACE_ACT:
                            nc.scalar.activation(
                                pa, pa, mybir.ActivationFunctionType.Sign,
                                bias=bias_sb, accum_out=act_sb[:, col:col + 1])
                        else:
                            sa = scr.tile([128, CA], bf16, tag="sa")
                            nc.scalar.activation(
                                sa, pa, mybir.ActivationFunctionType.Sign,
                                bias=bias_sb, accum_out=act_sb[:, col:col + 1])

                        if not DVE_FIRST:
                            pb = psB.tile([128, CD], f32)
                            fill_psum(pb, lt, base + CA, CD)
                            if INPLACE_ACT:
                                nc.vector.tensor_scalar(
                                    pb, pb, 0.0, 0.0,
                                    op0=mybir.AluOpType.is_ge, op1=mybir.AluOpType.add,
                                    accum_out=dve_sb[:, col:col + 1])
                            else:
                                sv = scr.tile([128, CD], f32, tag="sv")
                                nc.vector.tensor_scalar(
                                    sv, pb, 0.0, 0.0,
                                    op0=mybir.AluOpType.is_ge, op1=mybir.AluOpType.add,
                                    accum_out=dve_sb[:, col:col + 1])

            if repeat > 1:
                with tc.For_i(0, repeat, 1):
                    body()
            else:
                body()

            nc.sync.dma_start(out=act_d, in_=act_sb)
            nc.sync.dma_start(out=dve_d, in_=dve_sb)
    nc.compile()
    _PROGRAMS[repeat] = nc
    return nc


def kernel(pointcloud, pointcloud_padding):
    global LAST_RESULTS
    from concourse.bass_utils import run_bass_kernel_spmd

    pc = np.asarray(pointcloud, np.float32)
    pad = np.asarray(pointcloud_padding, np.float32)

    lhs = _build_lhs(pc)
    in_maps = [{"lhs_t": lhs, "rhs": _build_rhs(pad[i * MS:(i + 1) * MS])}
               for i in range(NCORES)]

    nc = _get_program()
    res = run_bass_kernel_spmd(nc, in_maps, core_ids=list(range(NCORES)))
    LAST_RESULTS = res

    total = np.zeros((128, NT), np.float32)
    for i in range(NCORES):
        A = res.results[i]["actsum"]
        D = res.results[i]["dvesum"]
        # Sign sums S over ACT_COLS valid +-1 entries: count = (S+ACT_COLS)/2
        total += (A[:, 0::2] + A[:, 1::2] + np.float32(ACT_COLS)) * np.float32(0.5)
        total += D[:, 0::2] + D[:, 1::2]
    counts = total.T.reshape(-1)[:N]
    return np.rint(counts).astype(np.int32).reshape(N, 1)

